# revision 4
# baseline (speedup 1.0000x reference)
"""Dual-normalized dot-product attention on 8 NeuronCores (Trainium2, Bass/Tile).

Problem (reference.py): B=64, L=1024, D=512 fp32.
  e = exp(A @ B^T);  e *= mask_a[:,None] * mask_b[None,:]
  a_att = (e / (sum_j e + eps)) @ B
  b_att = (e / (sum_i e + eps))^T @ A

Sharding: data-parallel over batch, 8 batches per core.

Per-core math per batch, arranged to reproduce the reference's NaN/Inf
pattern exactly (exp overflows to inf above ~88.72; inf*0 -> nan poisons
whole rows/cols exactly as in the reference):
  S   = A @ B^T            PE fp32 (4 K-chunks into PSUM)
  u   = exp(S)             ACT (same LUT the jax/axon reference uses)
  tm  = u * (mb * sigma)   GPSIMD tensor_tensor; mb bcast along partitions;
                           sigma=2^-64 prevents overflow downstream
  uu  = tm * ma            DVE tensor_scalar -> f32r, accum_out = rowsum
  va  = transpose(tm)      PE transpose (fp32) -> ACT copy -> f32r
  csum[j] = sum_i uu       PE matmuls vs ones vector (N=1)
  b_att[j] = (sum_i uu[i,j] * A_r[i]) * recip(csum_j + sigma*eps)
  a_att[i] = (sum_j va[j,i] * B_r[j]) * ma_i * recip(rowsum_i + sigma*eps)
Normalization/masks fold into per-partition output scales and uu, so all
inf/nan propagation follows IEEE semantics, matching the reference.
"""

import numpy as np

B_FULL = 64
N_CORES = 8
NB = B_FULL // N_CORES   # batches per core
L = 1024
D = 512
P = 128
IC = L // P              # 8 i-chunks
JC = L // P              # 8 j-chunks
KC = D // P              # 4 k-chunks (contraction for scores)
SIGMA = 2.0 ** -64
EPS = 1e-7

_compiled = None


def _build():
    import concourse.bacc as bacc
    import concourse.mybir as mybir
    import concourse.tile as tile

    F32 = mybir.dt.float32
    F32R = mybir.dt.float32r
    I32 = mybir.dt.int32
    MULT = mybir.AluOpType.mult
    ADD = mybir.AluOpType.add

    nc = bacc.Bacc("TRN2", target_bir_lowering=False, debug=False,
                   num_devices=N_CORES)

    A_D = nc.dram_tensor("inputs_a", [NB, L, D], F32, kind="ExternalInput").ap()
    B_D = nc.dram_tensor("inputs_b", [NB, L, D], F32, kind="ExternalInput").ap()
    MA_D = nc.dram_tensor("mask_a", [NB, L], I32, kind="ExternalInput").ap()
    MB_D = nc.dram_tensor("mask_b", [NB, L], I32, kind="ExternalInput").ap()
    IDENT_D = nc.dram_tensor("ident", [P, P], F32, kind="ExternalInput").ap()
    ONES_D = nc.dram_tensor("ones", [P, 2], F32R, kind="ExternalInput").ap()
    AATT_D = nc.dram_tensor("a_att", [NB, L, D], F32, kind="ExternalOutput").ap()
    BATT_D = nc.dram_tensor("b_att", [NB, L, D], F32, kind="ExternalOutput").ap()

    with tile.TileContext(nc) as tc:
        with (
            tc.tile_pool(name="const", bufs=1) as constp,
            tc.tile_pool(name="io", bufs=1) as iop,
            tc.tile_pool(name="big", bufs=1) as bigp,
            tc.tile_pool(name="mbp", bufs=1) as mbp,
            tc.tile_pool(name="tp", bufs=2) as tp,
            tc.tile_pool(name="outp", bufs=3) as outp,
            tc.tile_pool(name="tiny", bufs=2) as tinyp,
            tc.tile_pool(name="ps", bufs=6, space="PSUM") as ps,
            tc.tile_pool(name="csp", bufs=2, space="PSUM") as csp,
        ):
            ident = constp.tile([P, P], F32, tag="ident")
            nc.sync.dma_start(out=ident[:], in_=IDENT_D)
            ones_r = constp.tile([P, 2], F32R, tag="ones")
            nc.sync.dma_start(out=ones_r[:], in_=ONES_D)

            for b in range(NB):
                # ---------- P0: loads ----------
                a_nat = iop.tile([P, IC, D], F32, tag="a_nat")
                b_nat = iop.tile([P, JC, D], F32, tag="b_nat")
                nc.sync.dma_start(
                    out=a_nat[:], in_=A_D[b].rearrange("(c p) d -> p c d", p=P))
                nc.sync.dma_start(
                    out=b_nat[:], in_=B_D[b].rearrange("(c p) d -> p c d", p=P))

                ma_i = tinyp.tile([P, IC], I32, tag="ma_i")
                nc.sync.dma_start(
                    out=ma_i[:], in_=MA_D[b].rearrange("(c p) -> p c", p=P))
                ma_f = tinyp.tile([P, IC], F32, tag="ma_f")
                nc.vector.tensor_copy(ma_f[:], ma_i[:])

                # mask_b row: dma int32 bits into row 0 of mbb, convert*sigma
                # in place, broadcast to all partitions.
                mbb = mbp.tile([P, L], F32, tag="mbb")
                nc.sync.dma_start(out=mbb[0:1, :].bitcast(I32),
                                  in_=MB_D[b][None, :])
                nc.vector.tensor_scalar(
                    out=mbb[0:1, :], in0=mbb[0:1, :].bitcast(I32),
                    scalar1=SIGMA, scalar2=None, op0=MULT)
                nc.gpsimd.partition_broadcast(mbb[:], mbb[0:1, :])

                # ---------- P1: A^T/B^T (PE transpose) + f32r casts ----------
                a_tt = bigp.tile([P, KC, L], F32, tag="a_tt")   # A^T [d, i]
                b_tt = bigp.tile([P, KC, L], F32, tag="b_tt")   # B^T [d, j]
                for src, dst in ((a_nat, a_tt), (b_nat, b_tt)):
                    for kc in range(KC):
                        for cq in range(2):
                            pt = ps.tile([P, 512], F32, tag="ps")
                            for q in range(4):
                                c = cq * 4 + q
                                nc.tensor.transpose(
                                    pt[:, q * P:(q + 1) * P],
                                    src[:, c, kc * P:(kc + 1) * P],
                                    ident[:])
                            nc.scalar.copy(
                                dst[:, kc, cq * 512:(cq + 1) * 512], pt[:])

                a_r = bigp.tile([P, IC, D], F32R, tag="a_r")
                b_r = bigp.tile([P, JC, D], F32R, tag="b_r")
                for c in range(IC):
                    nc.vector.tensor_copy(a_r[:, c], a_nat[:, c])
                    nc.vector.tensor_copy(b_r[:, c], b_nat[:, c])

                # ---------- P2: scores -> exp -> mask -> transpose ----------
                uu = bigp.tile([P, IC, L], F32R, tag="uu")   # u*mb*sigma*ma [i,j]
                va = bigp.tile([P, JC, L], F32R, tag="va")   # (u*mb*sigma)^T [j,i]
                rs = tinyp.tile([P, IC], F32, tag="rs")      # rowsums
                cs_ps = csp.tile([P, 16], F32, tag="cs")     # colsum psum (dup pairs)
                for ic in range(IC):
                    sp0 = ps.tile([P, 512], F32, tag="ps")
                    sp1 = ps.tile([P, 512], F32, tag="ps")
                    for kc in range(KC):
                        nc.tensor.matmul(
                            sp0[:], a_tt[:, kc, ic * P:(ic + 1) * P],
                            b_tt[:, kc, 0:512],
                            start=(kc == 0), stop=(kc == KC - 1))
                    for kc in range(KC):
                        nc.tensor.matmul(
                            sp1[:], a_tt[:, kc, ic * P:(ic + 1) * P],
                            b_tt[:, kc, 512:1024],
                            start=(kc == 0), stop=(kc == KC - 1))
                    u_t = tp.tile([P, L], F32, tag="u_t")
                    nc.scalar.activation(u_t[:, 0:512], sp0[:],
                                         mybir.ActivationFunctionType.Exp)
                    nc.scalar.activation(u_t[:, 512:1024], sp1[:],
                                         mybir.ActivationFunctionType.Exp)
                    tm_t = tp.tile([P, L], F32, tag="tm_t")
                    nc.gpsimd.tensor_tensor(out=tm_t[:], in0=u_t[:], in1=mbb[:],
                                            op=MULT)
                    nc.vector.tensor_scalar(
                        out=uu[:, ic], in0=tm_t[:], scalar1=ma_f[:, ic:ic + 1],
                        scalar2=None, op0=MULT, op1=ADD,
                        accum_out=rs[:, ic:ic + 1])
                    # transpose tm (fp32) -> va (f32r via ACT rounding)
                    for cq in range(2):
                        pt = ps.tile([P, 512], F32, tag="ps")
                        for q in range(4):
                            jc = cq * 4 + q
                            nc.tensor.transpose(
                                pt[:, q * P:(q + 1) * P],
                                tm_t[:, jc * P:(jc + 1) * P],
                                ident[:])
                        nc.scalar.copy(
                            va[:, cq * 4:(cq + 1) * 4, ic * P:(ic + 1) * P],
                            pt[:].rearrange("p (q c) -> p q c", q=4))
                    # colsum: cs[:, jc] += uu[:, ic, jc-blk]^T @ ones
                    for jc in range(JC):
                        nc.tensor.matmul(
                            cs_ps[:, 2 * jc:2 * jc + 2],
                            uu[:, ic, jc * P:(jc + 1) * P], ones_r[:],
                            start=(ic == 0 and jc == 0),
                            stop=(ic == IC - 1 and jc == JC - 1),
                            skip_group_check=True)

                # ---------- scales ----------
                # The reference's fp32 row/col sums can overflow to inf even
                # when every entry is finite (sum > 3.4e38) -> its outputs
                # become exact 0 rows. Reproduce: scale our sigma-scaled sum
                # up by 2^64 (overflows to inf at exactly the fp32 boundary)
                # and back down (inf survives, finite values round-trip).
                cs_sb = tinyp.tile([P, JC], F32, tag="cs_sb")
                cs_ovf = tinyp.tile([P, JC], F32, tag="cs_ovf")
                cs_even = cs_ps[:].rearrange("p (c two) -> p c two", two=2)[:, :, 0]
                nc.vector.tensor_scalar(
                    out=cs_ovf[:], in0=cs_even, scalar1=2.0 ** 64,
                    scalar2=2.0 ** -64, op0=MULT, op1=MULT)
                nc.vector.tensor_scalar(
                    out=cs_sb[:], in0=cs_ovf[:], scalar1=SIGMA * EPS,
                    scalar2=None, op0=ADD)
                bscale = tinyp.tile([P, JC], F32, tag="bscale")
                nc.vector.reciprocal(bscale[:], cs_sb[:])

                rs_ovf = tinyp.tile([P, IC], F32, tag="rs_ovf")
                nc.vector.tensor_scalar(
                    out=rs_ovf[:], in0=rs[:], scalar1=2.0 ** 64,
                    scalar2=2.0 ** -64, op0=MULT, op1=MULT)
                rs_eps = tinyp.tile([P, IC], F32, tag="rs_eps")
                nc.vector.tensor_scalar(
                    out=rs_eps[:], in0=rs_ovf[:], scalar1=SIGMA * EPS,
                    scalar2=None, op0=ADD)
                arec = tinyp.tile([P, IC], F32, tag="arec")
                nc.vector.reciprocal(arec[:], rs_eps[:])
                ascale = tinyp.tile([P, IC], F32, tag="ascale")
                nc.vector.tensor_tensor(out=ascale[:], in0=arec[:], in1=ma_f[:],
                                        op=MULT)

                # ---------- P3: b_att ----------
                for jc in range(JC):
                    bp = ps.tile([P, D], F32, tag="ps")
                    for ic in range(IC):
                        nc.tensor.matmul(
                            bp[:], uu[:, ic, jc * P:(jc + 1) * P], a_r[:, ic],
                            start=(ic == 0), stop=(ic == IC - 1))
                    bo = outp.tile([P, D], F32, tag="ob")
                    nc.vector.tensor_scalar(
                        out=bo[:], in0=bp[:], scalar1=bscale[:, jc:jc + 1],
                        scalar2=None, op0=MULT)
                    nc.sync.dma_start(
                        out=BATT_D[b, jc * P:(jc + 1) * P, :], in_=bo[:])

                # ---------- P4: a_att ----------
                for ic in range(IC):
                    ap_ = ps.tile([P, D], F32, tag="ps")
                    for jc in range(JC):
                        nc.tensor.matmul(
                            ap_[:], va[:, jc, ic * P:(ic + 1) * P], b_r[:, jc],
                            start=(jc == 0), stop=(jc == JC - 1))
                    ao = outp.tile([P, D], F32, tag="ob")
                    nc.vector.tensor_scalar(
                        out=ao[:], in0=ap_[:], scalar1=ascale[:, ic:ic + 1],
                        scalar2=None, op0=MULT)
                    nc.sync.dma_start(
                        out=AATT_D[b, ic * P:(ic + 1) * P, :], in_=ao[:])

    nc.compile()
    return nc


def _get_compiled():
    global _compiled
    if _compiled is None:
        _compiled = _build()
    return _compiled


def kernel(inputs_a, inputs_b, mask_a, mask_b):
    from concourse.bass_utils import run_bass_kernel_spmd

    nc = _get_compiled()

    inputs_a = np.ascontiguousarray(inputs_a, dtype=np.float32)
    inputs_b = np.ascontiguousarray(inputs_b, dtype=np.float32)
    mask_a = np.ascontiguousarray(mask_a, dtype=np.int32)
    mask_b = np.ascontiguousarray(mask_b, dtype=np.int32)

    ident = np.eye(P, dtype=np.float32)
    ones = np.ones((P, 2), dtype=np.float32)

    in_maps = []
    for c in range(N_CORES):
        sl = slice(c * NB, (c + 1) * NB)
        in_maps.append({
            "inputs_a": inputs_a[sl],
            "inputs_b": inputs_b[sl],
            "mask_a": mask_a[sl],
            "mask_b": mask_b[sl],
            "ident": ident,
            "ones": ones,
        })

    res = run_bass_kernel_spmd(nc, in_maps, list(range(N_CORES)))
    a_att = np.concatenate([r["a_att"] for r in res.results], axis=0)
    b_att = np.concatenate([r["b_att"] for r in res.results], axis=0)
    return (a_att, b_att)


# revision 5
# speedup vs baseline: 12981.1696x; 12981.1696x over previous
"""Dual-normalized dot-product attention on 8 NeuronCores (Trainium2, Bass/Tile).

Problem (reference.py): B=64, L=1024, D=512 fp32.
  e = exp(A @ B^T);  e *= mask_a[:,None] * mask_b[None,:]
  a_att = (e / (sum_j e + eps)) @ B
  b_att = (e / (sum_i e + eps))^T @ A

Sharding: data-parallel over batch, 8 batches per core.

Per-core math per batch, arranged to reproduce the reference's NaN/Inf
pattern exactly (exp overflows to inf above ~88.72; inf*0 -> nan poisons
whole rows/cols exactly as in the reference):
  S   = A @ B^T            PE fp32 (4 K-chunks into PSUM)
  u   = exp(S)             ACT (same LUT the jax/axon reference uses)
  tm  = u * (mb * sigma)   GPSIMD tensor_tensor; mb bcast along partitions;
                           sigma=2^-64 prevents overflow downstream
  uu  = tm * ma            DVE tensor_scalar -> f32r, accum_out = rowsum
  va  = transpose(tm)      PE transpose (fp32) -> ACT copy -> f32r
  csum[j] = sum_i uu       PE matmuls vs ones vector (N=1)
  b_att[j] = (sum_i uu[i,j] * A_r[i]) * recip(csum_j + sigma*eps)
  a_att[i] = (sum_j va[j,i] * B_r[j]) * ma_i * recip(rowsum_i + sigma*eps)
Normalization/masks fold into per-partition output scales and uu, so all
inf/nan propagation follows IEEE semantics, matching the reference.
"""

import numpy as np

B_FULL = 64
N_CORES = 8
NB = B_FULL // N_CORES   # batches per core
L = 1024
D = 512
P = 128
IC = L // P              # 8 i-chunks
JC = L // P              # 8 j-chunks
KC = D // P              # 4 k-chunks (contraction for scores)
SIGMA = 2.0 ** -64
EPS = 1e-7

_compiled = None


def _build():
    import concourse.bacc as bacc
    import concourse.mybir as mybir
    import concourse.tile as tile

    F32 = mybir.dt.float32
    F32R = mybir.dt.float32r
    I32 = mybir.dt.int32
    MULT = mybir.AluOpType.mult
    ADD = mybir.AluOpType.add

    nc = bacc.Bacc("TRN2", target_bir_lowering=False, debug=False,
                   num_devices=N_CORES)

    A_D = nc.dram_tensor("inputs_a", [NB, L, D], F32, kind="ExternalInput").ap()
    B_D = nc.dram_tensor("inputs_b", [NB, L, D], F32, kind="ExternalInput").ap()
    MA_D = nc.dram_tensor("mask_a", [NB, L], I32, kind="ExternalInput").ap()
    MB_D = nc.dram_tensor("mask_b", [NB, L], I32, kind="ExternalInput").ap()
    IDENT_D = nc.dram_tensor("ident", [P, P], F32, kind="ExternalInput").ap()
    ONES_D = nc.dram_tensor("ones", [P, 2], F32R, kind="ExternalInput").ap()
    AATT_D = nc.dram_tensor("a_att", [NB, L, D], F32, kind="ExternalOutput").ap()
    BATT_D = nc.dram_tensor("b_att", [NB, L, D], F32, kind="ExternalOutput").ap()

    with tile.TileContext(nc) as tc:
        with (
            tc.tile_pool(name="const", bufs=1) as constp,
            tc.tile_pool(name="io", bufs=1) as iop,
            tc.tile_pool(name="big", bufs=1) as bigp,
            tc.tile_pool(name="mbp", bufs=1) as mbp,
            tc.tile_pool(name="tp", bufs=2) as tp,
            tc.tile_pool(name="outp", bufs=3) as outp,
            tc.tile_pool(name="tiny", bufs=2) as tinyp,
            tc.tile_pool(name="ps", bufs=6, space="PSUM") as ps,
            tc.tile_pool(name="csp", bufs=2, space="PSUM") as csp,
        ):
            ident = constp.tile([P, P], F32, tag="ident")
            nc.sync.dma_start(out=ident[:], in_=IDENT_D)
            ones_r = constp.tile([P, 2], F32R, tag="ones")
            nc.sync.dma_start(out=ones_r[:], in_=ONES_D)

            for b in range(NB):
                # ---------- P0: loads ----------
                a_nat = iop.tile([P, IC, D], F32, tag="a_nat")
                b_nat = iop.tile([P, JC, D], F32, tag="b_nat")
                nc.sync.dma_start(
                    out=a_nat[:], in_=A_D[b].rearrange("(c p) d -> p c d", p=P))
                nc.sync.dma_start(
                    out=b_nat[:], in_=B_D[b].rearrange("(c p) d -> p c d", p=P))

                ma_i = tinyp.tile([P, IC], I32, tag="ma_i")
                nc.sync.dma_start(
                    out=ma_i[:], in_=MA_D[b].rearrange("(c p) -> p c", p=P))
                ma_f = tinyp.tile([P, IC], F32, tag="ma_f")
                nc.vector.tensor_copy(ma_f[:], ma_i[:])

                # mask_b row: dma int32 bits into row 0 of mbb, convert*sigma
                # in place, broadcast to all partitions.
                mbb = mbp.tile([P, L], F32, tag="mbb")
                nc.sync.dma_start(out=mbb[0:1, :].bitcast(I32),
                                  in_=MB_D[b][None, :])
                nc.vector.tensor_scalar(
                    out=mbb[0:1, :], in0=mbb[0:1, :].bitcast(I32),
                    scalar1=SIGMA, scalar2=None, op0=MULT)
                nc.gpsimd.partition_broadcast(mbb[:], mbb[0:1, :])

                # ---------- P1: A^T/B^T (PE transpose) + f32r casts ----------
                a_tt = bigp.tile([P, KC, L], F32, tag="a_tt")   # A^T [d, i]
                b_tt = bigp.tile([P, KC, L], F32, tag="b_tt")   # B^T [d, j]
                for src, dst in ((a_nat, a_tt), (b_nat, b_tt)):
                    for kc in range(KC):
                        for cq in range(2):
                            pt = ps.tile([P, 512], F32, tag="ps")
                            for q in range(4):
                                c = cq * 4 + q
                                nc.tensor.transpose(
                                    pt[:, q * P:(q + 1) * P],
                                    src[:, c, kc * P:(kc + 1) * P],
                                    ident[:])
                            nc.scalar.copy(
                                dst[:, kc, cq * 512:(cq + 1) * 512], pt[:])

                a_r = bigp.tile([P, IC, D], F32R, tag="a_r")
                b_r = bigp.tile([P, JC, D], F32R, tag="b_r")
                for c in range(IC):
                    nc.vector.tensor_copy(a_r[:, c], a_nat[:, c])
                    nc.vector.tensor_copy(b_r[:, c], b_nat[:, c])

                # ---------- P2: scores -> exp -> mask -> transpose ----------
                uu = bigp.tile([P, IC, L], F32R, tag="uu")   # u*mb*sigma*ma [i,j]
                va = bigp.tile([P, JC, L], F32R, tag="va")   # (u*mb*sigma)^T [j,i]
                rs = tinyp.tile([P, IC], F32, tag="rs")      # rowsums
                cs_ps = csp.tile([P, 16], F32, tag="cs")     # colsum psum (dup pairs)
                # Software-pipelined: emit ic's transposes/colsums only after
                # ic+1's score matmuls, so the PE has dense work while the
                # exp(ACT) -> mask(GPSIMD) -> f32r-cast(DVE) chain for ic runs.
                def emit_scores(ic):
                    sp0 = ps.tile([P, 512], F32, tag="ps")
                    sp1 = ps.tile([P, 512], F32, tag="ps")
                    for kc in range(KC):
                        nc.tensor.matmul(
                            sp0[:], a_tt[:, kc, ic * P:(ic + 1) * P],
                            b_tt[:, kc, 0:512],
                            start=(kc == 0), stop=(kc == KC - 1))
                    for kc in range(KC):
                        nc.tensor.matmul(
                            sp1[:], a_tt[:, kc, ic * P:(ic + 1) * P],
                            b_tt[:, kc, 512:1024],
                            start=(kc == 0), stop=(kc == KC - 1))
                    return sp0, sp1

                def emit_mask_chain(ic, sp0, sp1):
                    u_t = tp.tile([P, L], F32, tag="u_t")
                    nc.scalar.activation(u_t[:, 0:512], sp0[:],
                                         mybir.ActivationFunctionType.Exp)
                    nc.scalar.activation(u_t[:, 512:1024], sp1[:],
                                         mybir.ActivationFunctionType.Exp)
                    tm_t = tp.tile([P, L], F32, tag="tm_t")
                    nc.gpsimd.tensor_tensor(out=tm_t[:], in0=u_t[:], in1=mbb[:],
                                            op=MULT)
                    nc.vector.tensor_scalar(
                        out=uu[:, ic], in0=tm_t[:], scalar1=ma_f[:, ic:ic + 1],
                        scalar2=None, op0=MULT, op1=ADD,
                        accum_out=rs[:, ic:ic + 1])
                    return tm_t

                def emit_tail(ic, tm_t):
                    # transpose tm (fp32) -> va (f32r via ACT rounding)
                    for cq in range(2):
                        pt = ps.tile([P, 512], F32, tag="ps")
                        for q in range(4):
                            jc = cq * 4 + q
                            nc.tensor.transpose(
                                pt[:, q * P:(q + 1) * P],
                                tm_t[:, jc * P:(jc + 1) * P],
                                ident[:])
                        nc.scalar.copy(
                            va[:, cq * 4:(cq + 1) * 4, ic * P:(ic + 1) * P],
                            pt[:].rearrange("p (q c) -> p q c", q=4))
                    # colsum: cs[:, 2jc] += uu[:, ic, jc-blk]^T @ ones
                    for jc in range(JC):
                        nc.tensor.matmul(
                            cs_ps[:, 2 * jc:2 * jc + 2],
                            uu[:, ic, jc * P:(jc + 1) * P], ones_r[:],
                            start=(ic == 0 and jc == 0),
                            stop=(ic == IC - 1 and jc == JC - 1),
                            skip_group_check=True)

                pend = None  # (ic, tm_t)
                for ic in range(IC):
                    sp0, sp1 = emit_scores(ic)
                    if pend is not None:
                        emit_tail(*pend)
                    tm_t = emit_mask_chain(ic, sp0, sp1)
                    pend = (ic, tm_t)
                emit_tail(*pend)

                # ---------- scales ----------
                # The reference's fp32 row/col sums can overflow to inf even
                # when every entry is finite (sum > 3.4e38) -> its outputs
                # become exact 0 rows. Reproduce: scale our sigma-scaled sum
                # up by 2^64 (overflows to inf at exactly the fp32 boundary)
                # and back down (inf survives, finite values round-trip).
                cs_sb = tinyp.tile([P, JC], F32, tag="cs_sb")
                cs_ovf = tinyp.tile([P, JC], F32, tag="cs_ovf")
                cs_even = cs_ps[:].rearrange("p (c two) -> p c two", two=2)[:, :, 0]
                nc.vector.tensor_scalar(
                    out=cs_ovf[:], in0=cs_even, scalar1=2.0 ** 64,
                    scalar2=2.0 ** -64, op0=MULT, op1=MULT)
                nc.vector.tensor_scalar(
                    out=cs_sb[:], in0=cs_ovf[:], scalar1=SIGMA * EPS,
                    scalar2=None, op0=ADD)
                bscale = tinyp.tile([P, JC], F32, tag="bscale")
                nc.vector.reciprocal(bscale[:], cs_sb[:])

                rs_ovf = tinyp.tile([P, IC], F32, tag="rs_ovf")
                nc.vector.tensor_scalar(
                    out=rs_ovf[:], in0=rs[:], scalar1=2.0 ** 64,
                    scalar2=2.0 ** -64, op0=MULT, op1=MULT)
                rs_eps = tinyp.tile([P, IC], F32, tag="rs_eps")
                nc.vector.tensor_scalar(
                    out=rs_eps[:], in0=rs_ovf[:], scalar1=SIGMA * EPS,
                    scalar2=None, op0=ADD)
                arec = tinyp.tile([P, IC], F32, tag="arec")
                nc.vector.reciprocal(arec[:], rs_eps[:])
                ascale = tinyp.tile([P, IC], F32, tag="ascale")
                nc.vector.tensor_tensor(out=ascale[:], in0=arec[:], in1=ma_f[:],
                                        op=MULT)

                # ---------- P3: b_att ----------
                for jc in range(JC):
                    bp = ps.tile([P, D], F32, tag="ps")
                    for ic in range(IC):
                        nc.tensor.matmul(
                            bp[:], uu[:, ic, jc * P:(jc + 1) * P], a_r[:, ic],
                            start=(ic == 0), stop=(ic == IC - 1))
                    bo = outp.tile([P, D], F32, tag="ob")
                    nc.vector.tensor_scalar(
                        out=bo[:], in0=bp[:], scalar1=bscale[:, jc:jc + 1],
                        scalar2=None, op0=MULT)
                    nc.sync.dma_start(
                        out=BATT_D[b, jc * P:(jc + 1) * P, :], in_=bo[:])

                # ---------- P4: a_att ----------
                for ic in range(IC):
                    ap_ = ps.tile([P, D], F32, tag="ps")
                    for jc in range(JC):
                        nc.tensor.matmul(
                            ap_[:], va[:, jc, ic * P:(ic + 1) * P], b_r[:, jc],
                            start=(jc == 0), stop=(jc == JC - 1))
                    ao = outp.tile([P, D], F32, tag="ob")
                    nc.vector.tensor_scalar(
                        out=ao[:], in0=ap_[:], scalar1=ascale[:, ic:ic + 1],
                        scalar2=None, op0=MULT)
                    nc.sync.dma_start(
                        out=AATT_D[b, ic * P:(ic + 1) * P, :], in_=ao[:])

    nc.compile()
    return nc


def _get_compiled():
    global _compiled
    if _compiled is None:
        _compiled = _build()
    return _compiled


def kernel(inputs_a, inputs_b, mask_a, mask_b):
    from concourse.bass_utils import run_bass_kernel_spmd

    nc = _get_compiled()

    inputs_a = np.ascontiguousarray(inputs_a, dtype=np.float32)
    inputs_b = np.ascontiguousarray(inputs_b, dtype=np.float32)
    mask_a = np.ascontiguousarray(mask_a, dtype=np.int32)
    mask_b = np.ascontiguousarray(mask_b, dtype=np.int32)

    ident = np.eye(P, dtype=np.float32)
    ones = np.ones((P, 2), dtype=np.float32)

    in_maps = []
    for c in range(N_CORES):
        sl = slice(c * NB, (c + 1) * NB)
        in_maps.append({
            "inputs_a": inputs_a[sl],
            "inputs_b": inputs_b[sl],
            "mask_a": mask_a[sl],
            "mask_b": mask_b[sl],
            "ident": ident,
            "ones": ones,
        })

    res = run_bass_kernel_spmd(nc, in_maps, list(range(N_CORES)))
    a_att = np.concatenate([r["a_att"] for r in res.results], axis=0)
    b_att = np.concatenate([r["b_att"] for r in res.results], axis=0)
    return (a_att, b_att)


# revision 18
# speedup vs baseline: 13555.6525x; 1.0443x over previous
"""Dual-normalized dot-product attention on 8 NeuronCores (Trainium2, Bass/Tile).

Problem (reference.py): B=64, L=1024, D=512 fp32.
  e = exp(A @ B^T);  e *= mask_a[:,None] * mask_b[None,:]
  a_att = (e / (sum_j e + eps)) @ B
  b_att = (e / (sum_i e + eps))^T @ A

Sharding: data-parallel over batch, 8 batches per core.

Per-core math per batch, arranged to reproduce the reference's NaN/Inf
pattern exactly (exp overflows to inf above ~88.72; inf*0 -> nan poisons
whole rows/cols exactly as in the reference):
  S   = A @ B^T            PE fp32 (4 K-chunks into PSUM)
  u   = exp(S)             ACT (same LUT the jax/axon reference uses)
  tm  = u * (mb * sigma)   GPSIMD tensor_tensor; mb bcast along partitions;
                           sigma=2^-64 prevents overflow downstream
  uu  = tm * ma            DVE tensor_scalar -> f32r, accum_out = rowsum
  va  = transpose(tm)      PE transpose (fp32) -> ACT copy -> f32r
  csum[j] = sum_i uu       PE matmuls vs ones vector (N=1)
  b_att[j] = (sum_i uu[i,j] * A_r[i]) * recip(csum_j + sigma*eps)
  a_att[i] = (sum_j va[j,i] * B_r[j]) * ma_i * recip(rowsum_i + sigma*eps)
Normalization/masks fold into per-partition output scales and uu, so all
inf/nan propagation follows IEEE semantics, matching the reference.
"""

import numpy as np

B_FULL = 64
N_CORES = 8
NB = B_FULL // N_CORES   # batches per core
L = 1024
D = 512
P = 128
IC = L // P              # 8 i-chunks
JC = L // P              # 8 j-chunks
KC = D // P              # 4 k-chunks (contraction for scores)
SIGMA = 2.0 ** -64
EPS = 1e-7

_compiled = None


def _build():
    import concourse.bacc as bacc
    import concourse.mybir as mybir
    import concourse.tile as tile

    F32 = mybir.dt.float32
    F32R = mybir.dt.float32r
    I32 = mybir.dt.int32
    MULT = mybir.AluOpType.mult
    ADD = mybir.AluOpType.add

    nc = bacc.Bacc("TRN2", target_bir_lowering=False, debug=False,
                   num_devices=N_CORES)

    A_D = nc.dram_tensor("inputs_a", [NB, L, D], F32, kind="ExternalInput").ap()
    B_D = nc.dram_tensor("inputs_b", [NB, L, D], F32, kind="ExternalInput").ap()
    MA_D = nc.dram_tensor("mask_a", [NB, L], I32, kind="ExternalInput").ap()
    MB_D = nc.dram_tensor("mask_b", [NB, L], I32, kind="ExternalInput").ap()
    IDENT_D = nc.dram_tensor("ident", [P, P], F32, kind="ExternalInput").ap()
    IDENTR_D = nc.dram_tensor("ident_r", [P, P], F32R, kind="ExternalInput").ap()
    ONES_D = nc.dram_tensor("ones", [P, 2], F32R, kind="ExternalInput").ap()
    AATT_D = nc.dram_tensor("a_att", [NB, L, D], F32, kind="ExternalOutput").ap()
    BATT_D = nc.dram_tensor("b_att", [NB, L, D], F32, kind="ExternalOutput").ap()

    with tile.TileContext(nc) as tc:
        with (
            tc.tile_pool(name="const", bufs=1) as constp,
            tc.tile_pool(name="io", bufs=1) as iop,
            tc.tile_pool(name="big", bufs=1) as bigp,
            tc.tile_pool(name="mbp", bufs=1) as mbp,
            tc.tile_pool(name="tp", bufs=2) as tp,
            tc.tile_pool(name="outp", bufs=3) as outp,
            tc.tile_pool(name="tiny", bufs=2) as tinyp,
            tc.tile_pool(name="ps", bufs=6, space="PSUM") as ps,
            tc.tile_pool(name="csp", bufs=2, space="PSUM") as csp,
        ):
            ident = constp.tile([P, P], F32, tag="ident")
            nc.sync.dma_start(out=ident[:], in_=IDENT_D)
            ident_r = constp.tile([P, P], F32R, tag="ident_r")
            nc.sync.dma_start(out=ident_r[:], in_=IDENTR_D)
            ones_r = constp.tile([P, 2], F32R, tag="ones")
            nc.sync.dma_start(out=ones_r[:], in_=ONES_D)

            for b in range(NB):
                # ---------- P0: loads ----------
                a_nat = iop.tile([P, IC, D], F32, tag="a_nat")
                b_nat = iop.tile([P, JC, D], F32, tag="b_nat")
                a_src = A_D[b].rearrange("(c p) d -> p c d", p=P)
                b_src = B_D[b].rearrange("(c p) d -> p c d", p=P)
                for c in range(IC):
                    nc.sync.dma_start(out=a_nat[:, c], in_=a_src[:, c])
                    nc.sync.dma_start(out=b_nat[:, c], in_=b_src[:, c])

                ma_i = tinyp.tile([P, IC], I32, tag="ma_i")
                nc.sync.dma_start(
                    out=ma_i[:], in_=MA_D[b].rearrange("(c p) -> p c", p=P))
                ma_f = tinyp.tile([P, IC], F32, tag="ma_f")
                nc.vector.tensor_copy(ma_f[:], ma_i[:])

                # mask_b row: dma int32 bits into row 0 of mbb, convert*sigma
                # in place, broadcast to all partitions.
                mbb = mbp.tile([P, L], F32, tag="mbb")
                nc.sync.dma_start(out=mbb[0:1, :].bitcast(I32),
                                  in_=MB_D[b][None, :])
                nc.vector.tensor_scalar(
                    out=mbb[0:1, :], in0=mbb[0:1, :].bitcast(I32),
                    scalar1=SIGMA, scalar2=None, op0=MULT)
                nc.gpsimd.partition_broadcast(mbb[:], mbb[0:1, :])

                # ---------- P1: A^T/B^T (PE transpose) + f32r casts ----------
                a_tt = bigp.tile([P, KC, L], F32, tag="a_tt")   # A^T [d, i]
                b_tt = bigp.tile([P, KC, L], F32, tag="b_tt")   # B^T [d, j]
                for src, dst in ((a_nat, a_tt), (b_nat, b_tt)):
                    for kc in range(KC):
                        for cq in range(2):
                            pt = ps.tile([P, 512], F32, tag="ps")
                            for q in range(4):
                                c = cq * 4 + q
                                nc.tensor.transpose(
                                    pt[:, q * P:(q + 1) * P],
                                    src[:, c, kc * P:(kc + 1) * P],
                                    ident[:])
                            nc.scalar.copy(
                                dst[:, kc, cq * 512:(cq + 1) * 512], pt[:])

                a_r = bigp.tile([P, IC, D], F32R, tag="a_r")
                b_r = bigp.tile([P, JC, D], F32R, tag="b_r")
                for c in range(IC):
                    nc.vector.tensor_copy(a_r[:, c], a_nat[:, c])
                    nc.vector.tensor_copy(b_r[:, c], b_nat[:, c])

                # ---------- P2: scores -> exp -> mask -> transpose ----------
                uu = bigp.tile([P, IC, L], F32R, tag="uu")   # u*mb*sigma*ma [i,j]
                va = bigp.tile([P, JC, L], F32R, tag="va")   # (u*mb*sigma)^T [j,i]
                rs = tinyp.tile([P, IC], F32, tag="rs")      # rowsums
                cs_ps = csp.tile([P, 16], F32, tag="cs")     # colsum psum (dup pairs)
                # Software-pipelined: emit ic's transposes/colsums only after
                # ic+1's score matmuls, so the PE has dense work while the
                # exp(ACT) -> mask(GPSIMD) -> f32r-cast(DVE) chain for ic runs.
                def emit_scores(ic):
                    sp0 = ps.tile([P, 512], F32, tag="ps")
                    sp1 = ps.tile([P, 512], F32, tag="ps")
                    for kc in range(KC):
                        nc.tensor.matmul(
                            sp0[:], a_tt[:, kc, ic * P:(ic + 1) * P],
                            b_tt[:, kc, 0:512],
                            start=(kc == 0), stop=(kc == KC - 1))
                    for kc in range(KC):
                        nc.tensor.matmul(
                            sp1[:], a_tt[:, kc, ic * P:(ic + 1) * P],
                            b_tt[:, kc, 512:1024],
                            start=(kc == 0), stop=(kc == KC - 1))
                    return sp0, sp1

                def emit_mask_chain(ic, sp0, sp1):
                    u_t = tp.tile([P, L], F32, tag="u_t")
                    nc.scalar.activation(u_t[:, 0:512], sp0[:],
                                         mybir.ActivationFunctionType.Exp)
                    nc.scalar.activation(u_t[:, 512:1024], sp1[:],
                                         mybir.ActivationFunctionType.Exp)
                    tm_t = tp.tile([P, L], F32, tag="tm_t")
                    nc.gpsimd.tensor_tensor(out=tm_t[:], in0=u_t[:], in1=mbb[:],
                                            op=MULT)
                    nc.vector.tensor_scalar(
                        out=uu[:, ic], in0=tm_t[:], scalar1=ma_f[:, ic:ic + 1],
                        scalar2=None, op0=MULT, op1=ADD,
                        accum_out=rs[:, ic:ic + 1])

                def emit_tail(ic):
                    # transpose uu (f32r, 1.5 cyc/row) -> va; ACT copy rounds
                    # the psum values back to f32r. uu carries mask_a, which
                    # is harmless for a_att (its rows fold into the ma-scaled
                    # output scale; nan rows stay nan).
                    for cq in range(2):
                        pt = ps.tile([P, 512], F32R, tag="ps")
                        for q in range(4):
                            jc = cq * 4 + q
                            nc.tensor.transpose(
                                pt[:, q * P:(q + 1) * P],
                                uu[:, ic, jc * P:(jc + 1) * P],
                                ident_r[:])
                        nc.scalar.copy(
                            va[:, cq * 4:(cq + 1) * 4, ic * P:(ic + 1) * P],
                            pt[:].rearrange("p (q c) -> p q c", q=4))
                    # colsum: cs[:, 2jc] += uu[:, ic, jc-blk]^T @ ones
                    for jc in range(JC):
                        nc.tensor.matmul(
                            cs_ps[:, 2 * jc:2 * jc + 2],
                            uu[:, ic, jc * P:(jc + 1) * P], ones_r[:],
                            start=(ic == 0 and jc == 0),
                            stop=(ic == IC - 1 and jc == JC - 1),
                            skip_group_check=True)

                pend = None
                for ic in range(IC):
                    sp0, sp1 = emit_scores(ic)
                    if pend is not None:
                        emit_tail(pend)
                    emit_mask_chain(ic, sp0, sp1)
                    pend = ic
                # The final tail is emitted inside P3 (after the first b_att
                # accumulation group) so the PE has work while ic=IC-1's
                # exp -> mask -> cast chain completes.

                # ---------- scales ----------
                # The reference's fp32 row/col sums can overflow to inf even
                # when every entry is finite (sum > 3.4e38) -> its outputs
                # become exact 0 rows. Reproduce: scale our sigma-scaled sum
                # up by 2^64 (overflows to inf at exactly the fp32 boundary)
                # and back down (inf survives, finite values round-trip).
                # (b-side scales are emitted inside P3, after the final
                # colsum matmuls of the deferred tail.)
                def emit_bscale():
                    cs_sb = tinyp.tile([P, JC], F32, tag="cs_sb")
                    cs_ovf = tinyp.tile([P, JC], F32, tag="cs_ovf")
                    cs_even = cs_ps[:].rearrange(
                        "p (c two) -> p c two", two=2)[:, :, 0]
                    nc.vector.tensor_scalar(
                        out=cs_ovf[:], in0=cs_even, scalar1=2.0 ** 64,
                        scalar2=2.0 ** -64, op0=MULT, op1=MULT)
                    nc.vector.tensor_scalar(
                        out=cs_sb[:], in0=cs_ovf[:], scalar1=SIGMA * EPS,
                        scalar2=None, op0=ADD)
                    bscale = tinyp.tile([P, JC], F32, tag="bscale")
                    nc.vector.reciprocal(bscale[:], cs_sb[:])
                    return bscale

                rs_ovf = tinyp.tile([P, IC], F32, tag="rs_ovf")
                nc.vector.tensor_scalar(
                    out=rs_ovf[:], in0=rs[:], scalar1=2.0 ** 64,
                    scalar2=2.0 ** -64, op0=MULT, op1=MULT)
                rs_eps = tinyp.tile([P, IC], F32, tag="rs_eps")
                nc.vector.tensor_scalar(
                    out=rs_eps[:], in0=rs_ovf[:], scalar1=SIGMA * EPS,
                    scalar2=None, op0=ADD)
                arec = tinyp.tile([P, IC], F32, tag="arec")
                nc.vector.reciprocal(arec[:], rs_eps[:])
                ascale = tinyp.tile([P, IC], F32, tag="ascale")
                nc.vector.tensor_tensor(out=ascale[:], in0=arec[:], in1=ma_f[:],
                                        op=MULT)

                # ---------- P3: b_att ----------
                # jc=0/1 groups run their first IC-1 accumulations before the
                # deferred tail so the PE stays busy while ic=IC-1's
                # exp -> mask -> cast chain (which the tail depends on) runs.
                def bmm(bp, jc, ic, start, stop):
                    nc.tensor.matmul(
                        bp[:], uu[:, ic, jc * P:(jc + 1) * P], a_r[:, ic],
                        start=start, stop=stop, skip_group_check=True)

                def emit_bout(bp, jc, bscale):
                    bo = outp.tile([P, D], F32, tag="ob")
                    nc.vector.tensor_scalar(
                        out=bo[:], in0=bp[:], scalar1=bscale[:, jc:jc + 1],
                        scalar2=None, op0=MULT)
                    nc.sync.dma_start(
                        out=BATT_D[b, jc * P:(jc + 1) * P, :], in_=bo[:])

                bps = {}
                for jc in (0, 1):
                    bp_pre = ps.tile([P, D], F32, tag="ps")
                    bps[jc] = bp_pre
                    for ic in range(IC - 1):
                        bmm(bps[jc], jc, ic, ic == 0, False)
                emit_tail(pend)
                pend = None
                bscale = emit_bscale()
                for jc in (0, 1):
                    bmm(bps[jc], jc, IC - 1, False, True)
                    emit_bout(bps[jc], jc, bscale)
                for jc in range(2, JC):
                    bp = ps.tile([P, D], F32, tag="ps")
                    for ic in range(IC):
                        bmm(bp, jc, ic, ic == 0, ic == IC - 1)
                    emit_bout(bp, jc, bscale)

                # ---------- P4: a_att ----------
                for ic in range(IC):
                    ap_ = ps.tile([P, D], F32, tag="ps")
                    for jc in range(JC):
                        nc.tensor.matmul(
                            ap_[:], va[:, jc, ic * P:(ic + 1) * P], b_r[:, jc],
                            start=(jc == 0), stop=(jc == JC - 1))
                    ao = outp.tile([P, D], F32, tag="ob")
                    nc.vector.tensor_scalar(
                        out=ao[:], in0=ap_[:], scalar1=ascale[:, ic:ic + 1],
                        scalar2=None, op0=MULT)
                    nc.sync.dma_start(
                        out=AATT_D[b, ic * P:(ic + 1) * P, :], in_=ao[:])

    nc.compile()
    return nc


def _get_compiled():
    global _compiled
    if _compiled is None:
        _compiled = _build()
    return _compiled


def kernel(inputs_a, inputs_b, mask_a, mask_b):
    from concourse.bass_utils import run_bass_kernel_spmd

    nc = _get_compiled()

    inputs_a = np.ascontiguousarray(inputs_a, dtype=np.float32)
    inputs_b = np.ascontiguousarray(inputs_b, dtype=np.float32)
    mask_a = np.ascontiguousarray(mask_a, dtype=np.int32)
    mask_b = np.ascontiguousarray(mask_b, dtype=np.int32)

    ident = np.eye(P, dtype=np.float32)
    ones = np.ones((P, 2), dtype=np.float32)

    in_maps = []
    for c in range(N_CORES):
        sl = slice(c * NB, (c + 1) * NB)
        in_maps.append({
            "inputs_a": inputs_a[sl],
            "inputs_b": inputs_b[sl],
            "mask_a": mask_a[sl],
            "mask_b": mask_b[sl],
            "ident": ident,
            "ident_r": ident,
            "ones": ones,
        })

    res = run_bass_kernel_spmd(nc, in_maps, list(range(N_CORES)))
    a_att = np.concatenate([r["a_att"] for r in res.results], axis=0)
    b_att = np.concatenate([r["b_att"] for r in res.results], axis=0)
    return (a_att, b_att)


# revision 28
# speedup vs baseline: 13674.4228x; 1.0088x over previous
"""Dual-normalized dot-product attention on 8 NeuronCores (Trainium2, Bass/Tile).

Problem (reference.py): B=64, L=1024, D=512 fp32.
  e = exp(A @ B^T);  e *= mask_a[:,None] * mask_b[None,:]
  a_att = (e / (sum_j e + eps)) @ B
  b_att = (e / (sum_i e + eps))^T @ A

Sharding: data-parallel over batch, 8 batches per core.

Per-core math per batch, arranged to reproduce the reference's NaN/Inf
pattern exactly (exp overflows to inf above ~88.72; inf*0 -> nan poisons
whole rows/cols exactly as in the reference):
  S   = A @ B^T            PE fp32 (4 K-chunks into PSUM)
  u   = exp(S)             ACT (same LUT the jax/axon reference uses)
  tm  = u * (mb * sigma)   GPSIMD tensor_tensor; mb bcast along partitions;
                           sigma=2^-64 prevents overflow downstream
  uu  = tm * ma            DVE tensor_scalar -> f32r, accum_out = rowsum
  va  = transpose(uu)      PE transpose (f32r, 1.5 cyc/row) -> ACT copy
  csum[j] = sum_i uu       PE matmuls vs ones vector (N=2, f32r dst rule)
  b_att[j] = (sum_i uu[i,j] * A_r[i]) * recip(csum_j + sigma*eps)
  a_att[i] = (sum_j va[j,i] * B_r[j]) * ma_i * recip(rowsum_i + sigma*eps)
Normalization/masks fold into per-partition output scales and uu, so all
inf/nan propagation follows IEEE semantics, matching the reference.
Row/col sums are pushed through a *2^64,*2^-64 fp32 round-trip so sums
that overflow fp32 in the reference become inf here too (-> exact 0 rows).

Scheduling: the score->exp->mask chain is software-pipelined across
i-chunks; the last chunk's transpose/colsum tail is deferred into P3
behind two b_att accumulation groups, keeping the PE ~98% busy
(cost-model timeline: 769 us/core, PE-bound by the fp32 score matmuls).
"""

import numpy as np

B_FULL = 64
N_CORES = 8
NB = B_FULL // N_CORES   # batches per core
L = 1024
D = 512
P = 128
IC = L // P              # 8 i-chunks
JC = L // P              # 8 j-chunks
KC = D // P              # 4 k-chunks (contraction for scores)
SIGMA = 2.0 ** -64
EPS = 1e-7

_compiled = None


def _build():
    import concourse.bacc as bacc
    import concourse.mybir as mybir
    import concourse.tile as tile

    F32 = mybir.dt.float32
    F32R = mybir.dt.float32r
    I32 = mybir.dt.int32
    MULT = mybir.AluOpType.mult
    ADD = mybir.AluOpType.add

    nc = bacc.Bacc("TRN2", target_bir_lowering=False, debug=False,
                   num_devices=N_CORES)

    A_D = nc.dram_tensor("inputs_a", [NB, L, D], F32, kind="ExternalInput").ap()
    B_D = nc.dram_tensor("inputs_b", [NB, L, D], F32, kind="ExternalInput").ap()
    MA_D = nc.dram_tensor("mask_a", [NB, L], I32, kind="ExternalInput").ap()
    MB_D = nc.dram_tensor("mask_b", [NB, L], I32, kind="ExternalInput").ap()
    IDENT_D = nc.dram_tensor("ident", [P, P], F32, kind="ExternalInput").ap()
    IDENTR_D = nc.dram_tensor("ident_r", [P, P], F32R, kind="ExternalInput").ap()
    ONES_D = nc.dram_tensor("ones", [P, 2], F32R, kind="ExternalInput").ap()
    AATT_D = nc.dram_tensor("a_att", [NB, L, D], F32, kind="ExternalOutput").ap()
    BATT_D = nc.dram_tensor("b_att", [NB, L, D], F32, kind="ExternalOutput").ap()

    with tile.TileContext(nc) as tc:
        with (
            tc.tile_pool(name="const", bufs=1) as constp,
            tc.tile_pool(name="io", bufs=1) as iop,
            tc.tile_pool(name="big", bufs=1) as bigp,
            tc.tile_pool(name="mbp", bufs=1) as mbp,
            tc.tile_pool(name="tp", bufs=2) as tp,
            tc.tile_pool(name="outp", bufs=3) as outp,
            tc.tile_pool(name="tiny", bufs=2) as tinyp,
            tc.tile_pool(name="ps", bufs=7, space="PSUM") as ps,
            tc.tile_pool(name="csp", bufs=1, space="PSUM") as csp,
        ):
            ident = constp.tile([P, P], F32, tag="ident")
            nc.sync.dma_start(out=ident[:], in_=IDENT_D)
            ident_r = constp.tile([P, P], F32R, tag="ident_r")
            nc.sync.dma_start(out=ident_r[:], in_=IDENTR_D)
            ones_r = constp.tile([P, 2], F32R, tag="ones")
            nc.sync.dma_start(out=ones_r[:], in_=ONES_D)

            for b in range(NB):
                # ---------- P0: loads ----------
                a_nat = iop.tile([P, IC, D], F32, tag="a_nat")
                b_nat = iop.tile([P, JC, D], F32, tag="b_nat")
                # A chunks first: the first P1 transposes only need A.
                a_src = A_D[b].rearrange("(c p) d -> p c d", p=P)
                b_src = B_D[b].rearrange("(c p) d -> p c d", p=P)
                for c in range(IC):
                    nc.sync.dma_start(out=a_nat[:, c], in_=a_src[:, c])
                for c in range(IC):
                    nc.sync.dma_start(out=b_nat[:, c], in_=b_src[:, c])

                ma_i = tinyp.tile([P, IC], I32, tag="ma_i")
                nc.sync.dma_start(
                    out=ma_i[:], in_=MA_D[b].rearrange("(c p) -> p c", p=P))
                ma_f = tinyp.tile([P, IC], F32, tag="ma_f")
                nc.vector.tensor_copy(ma_f[:], ma_i[:])

                # mask_b row: dma int32 bits into row 0 of mbb, convert*sigma
                # in place, broadcast to all partitions.
                mbb = mbp.tile([P, L], F32, tag="mbb")
                nc.sync.dma_start(out=mbb[0:1, :].bitcast(I32),
                                  in_=MB_D[b][None, :])
                nc.vector.tensor_scalar(
                    out=mbb[0:1, :], in0=mbb[0:1, :].bitcast(I32),
                    scalar1=SIGMA, scalar2=None, op0=MULT)
                nc.gpsimd.partition_broadcast(mbb[:], mbb[0:1, :])

                # ---------- P1: A^T/B^T (PE transpose) + f32r casts ----------
                a_tt = bigp.tile([P, KC, L], F32, tag="a_tt")   # A^T [d, i]
                b_tt = bigp.tile([P, KC, L], F32, tag="b_tt")   # B^T [d, j]
                for src, dst in ((a_nat, a_tt), (b_nat, b_tt)):
                    for kc in range(KC):
                        for cq in range(2):
                            pt = ps.tile([P, 512], F32, tag="ps")
                            for q in range(4):
                                c = cq * 4 + q
                                nc.tensor.transpose(
                                    pt[:, q * P:(q + 1) * P],
                                    src[:, c, kc * P:(kc + 1) * P],
                                    ident[:])
                            nc.scalar.copy(
                                dst[:, kc, cq * 512:(cq + 1) * 512], pt[:])

                a_r = bigp.tile([P, IC, D], F32R, tag="a_r")
                b_r = bigp.tile([P, JC, D], F32R, tag="b_r")
                for c in range(IC):
                    nc.vector.tensor_copy(a_r[:, c], a_nat[:, c])
                    nc.vector.tensor_copy(b_r[:, c], b_nat[:, c])

                # ---------- P2: scores -> exp -> mask -> transpose ----------
                uu = bigp.tile([P, IC, L], F32R, tag="uu")   # u*mb*sigma*ma [i,j]
                va = bigp.tile([P, JC, L], F32R, tag="va")   # (u*mb*sigma)^T [j,i]
                rs = tinyp.tile([P, IC], F32, tag="rs")      # rowsums
                cs_ps = csp.tile([P, 16], F32, tag="cs")     # colsum psum (dup pairs)
                # Software-pipelined: emit ic's transposes/colsums only after
                # ic+1's score matmuls, so the PE has dense work while the
                # exp(ACT) -> mask(GPSIMD) -> f32r-cast(DVE) chain for ic runs.
                def emit_scores(ic):
                    sp0 = ps.tile([P, 512], F32, tag="ps")
                    sp1 = ps.tile([P, 512], F32, tag="ps")
                    for kc in range(KC):
                        nc.tensor.matmul(
                            sp0[:], a_tt[:, kc, ic * P:(ic + 1) * P],
                            b_tt[:, kc, 0:512],
                            start=(kc == 0), stop=(kc == KC - 1))
                    for kc in range(KC):
                        nc.tensor.matmul(
                            sp1[:], a_tt[:, kc, ic * P:(ic + 1) * P],
                            b_tt[:, kc, 512:1024],
                            start=(kc == 0), stop=(kc == KC - 1))
                    return sp0, sp1

                def emit_mask_chain(ic, sp0, sp1):
                    u_t = tp.tile([P, L], F32, tag="u_t")
                    nc.scalar.activation(u_t[:, 0:512], sp0[:],
                                         mybir.ActivationFunctionType.Exp)
                    nc.scalar.activation(u_t[:, 512:1024], sp1[:],
                                         mybir.ActivationFunctionType.Exp)
                    tm_t = tp.tile([P, L], F32, tag="tm_t")
                    nc.gpsimd.tensor_tensor(out=tm_t[:], in0=u_t[:], in1=mbb[:],
                                            op=MULT)
                    nc.vector.tensor_scalar(
                        out=uu[:, ic], in0=tm_t[:], scalar1=ma_f[:, ic:ic + 1],
                        scalar2=None, op0=MULT, op1=ADD,
                        accum_out=rs[:, ic:ic + 1])

                def emit_tail(ic):
                    # transpose uu (f32r, 1.5 cyc/row) -> va; ACT copy rounds
                    # the psum values back to f32r. uu carries mask_a, which
                    # is harmless for a_att (its rows fold into the ma-scaled
                    # output scale; nan rows stay nan).
                    for cq in range(2):
                        pt = ps.tile([P, 512], F32R, tag="ps")
                        for q in range(4):
                            jc = cq * 4 + q
                            nc.tensor.transpose(
                                pt[:, q * P:(q + 1) * P],
                                uu[:, ic, jc * P:(jc + 1) * P],
                                ident_r[:])
                        nc.scalar.copy(
                            va[:, cq * 4:(cq + 1) * 4, ic * P:(ic + 1) * P],
                            pt[:].rearrange("p (q c) -> p q c", q=4))
                    # colsum: cs[:, 2jc] += uu[:, ic, jc-blk]^T @ ones
                    for jc in range(JC):
                        nc.tensor.matmul(
                            cs_ps[:, 2 * jc:2 * jc + 2],
                            uu[:, ic, jc * P:(jc + 1) * P], ones_r[:],
                            start=(ic == 0 and jc == 0),
                            stop=(ic == IC - 1 and jc == JC - 1),
                            skip_group_check=True)

                pend = None
                for ic in range(IC):
                    sp0, sp1 = emit_scores(ic)
                    if pend is not None:
                        emit_tail(pend)
                    emit_mask_chain(ic, sp0, sp1)
                    pend = ic
                # The final tail is emitted inside P3 (after the first b_att
                # accumulation group) so the PE has work while ic=IC-1's
                # exp -> mask -> cast chain completes.

                # ---------- scales ----------
                # The reference's fp32 row/col sums can overflow to inf even
                # when every entry is finite (sum > 3.4e38) -> its outputs
                # become exact 0 rows. Reproduce: scale our sigma-scaled sum
                # up by 2^64 (overflows to inf at exactly the fp32 boundary)
                # and back down (inf survives, finite values round-trip).
                # (b-side scales are emitted inside P3, after the final
                # colsum matmuls of the deferred tail.)
                def emit_bscale():
                    cs_sb = tinyp.tile([P, JC], F32, tag="cs_sb")
                    cs_ovf = tinyp.tile([P, JC], F32, tag="cs_ovf")
                    cs_even = cs_ps[:].rearrange(
                        "p (c two) -> p c two", two=2)[:, :, 0]
                    nc.vector.tensor_scalar(
                        out=cs_ovf[:], in0=cs_even, scalar1=2.0 ** 64,
                        scalar2=2.0 ** -64, op0=MULT, op1=MULT)
                    nc.vector.tensor_scalar(
                        out=cs_sb[:], in0=cs_ovf[:], scalar1=SIGMA * EPS,
                        scalar2=None, op0=ADD)
                    bscale = tinyp.tile([P, JC], F32, tag="bscale")
                    nc.vector.reciprocal(bscale[:], cs_sb[:])
                    return bscale

                rs_ovf = tinyp.tile([P, IC], F32, tag="rs_ovf")
                nc.vector.tensor_scalar(
                    out=rs_ovf[:], in0=rs[:], scalar1=2.0 ** 64,
                    scalar2=2.0 ** -64, op0=MULT, op1=MULT)
                rs_eps = tinyp.tile([P, IC], F32, tag="rs_eps")
                nc.vector.tensor_scalar(
                    out=rs_eps[:], in0=rs_ovf[:], scalar1=SIGMA * EPS,
                    scalar2=None, op0=ADD)
                arec = tinyp.tile([P, IC], F32, tag="arec")
                nc.vector.reciprocal(arec[:], rs_eps[:])
                ascale = tinyp.tile([P, IC], F32, tag="ascale")
                nc.vector.tensor_tensor(out=ascale[:], in0=arec[:], in1=ma_f[:],
                                        op=MULT)

                # ---------- P3: b_att ----------
                # jc=0/1 groups run their first IC-1 accumulations before the
                # deferred tail so the PE stays busy while ic=IC-1's
                # exp -> mask -> cast chain (which the tail depends on) runs.
                def bmm(bp, jc, ic, start, stop):
                    nc.tensor.matmul(
                        bp[:], uu[:, ic, jc * P:(jc + 1) * P], a_r[:, ic],
                        start=start, stop=stop, skip_group_check=True)

                def emit_bout(bp, jc, bscale):
                    bo = outp.tile([P, D], F32, tag="ob")
                    nc.vector.tensor_scalar(
                        out=bo[:], in0=bp[:], scalar1=bscale[:, jc:jc + 1],
                        scalar2=None, op0=MULT)
                    nc.sync.dma_start(
                        out=BATT_D[b, jc * P:(jc + 1) * P, :], in_=bo[:])

                bps = {}
                for jc in (0, 1):
                    bp_pre = ps.tile([P, D], F32, tag="ps")
                    bps[jc] = bp_pre
                    for ic in range(IC - 1):
                        bmm(bps[jc], jc, ic, ic == 0, False)
                emit_tail(pend)
                pend = None
                bscale = emit_bscale()
                for jc in (0, 1):
                    bmm(bps[jc], jc, IC - 1, False, True)
                    emit_bout(bps[jc], jc, bscale)
                for jc in range(2, JC):
                    bp = ps.tile([P, D], F32, tag="ps")
                    for ic in range(IC):
                        bmm(bp, jc, ic, ic == 0, ic == IC - 1)
                    emit_bout(bp, jc, bscale)

                # ---------- P4: a_att ----------
                for ic in range(IC):
                    ap_ = ps.tile([P, D], F32, tag="ps")
                    for jc in range(JC):
                        nc.tensor.matmul(
                            ap_[:], va[:, jc, ic * P:(ic + 1) * P], b_r[:, jc],
                            start=(jc == 0), stop=(jc == JC - 1))
                    ao = outp.tile([P, D], F32, tag="ob")
                    nc.vector.tensor_scalar(
                        out=ao[:], in0=ap_[:], scalar1=ascale[:, ic:ic + 1],
                        scalar2=None, op0=MULT)
                    nc.sync.dma_start(
                        out=AATT_D[b, ic * P:(ic + 1) * P, :], in_=ao[:])

    nc.compile()
    return nc


def _get_compiled():
    global _compiled
    if _compiled is None:
        _compiled = _build()
    return _compiled


def kernel(inputs_a, inputs_b, mask_a, mask_b):
    from concourse.bass_utils import run_bass_kernel_spmd

    nc = _get_compiled()

    inputs_a = np.ascontiguousarray(inputs_a, dtype=np.float32)
    inputs_b = np.ascontiguousarray(inputs_b, dtype=np.float32)
    mask_a = np.ascontiguousarray(mask_a, dtype=np.int32)
    mask_b = np.ascontiguousarray(mask_b, dtype=np.int32)

    ident = np.eye(P, dtype=np.float32)
    ones = np.ones((P, 2), dtype=np.float32)

    in_maps = []
    for c in range(N_CORES):
        sl = slice(c * NB, (c + 1) * NB)
        in_maps.append({
            "inputs_a": inputs_a[sl],
            "inputs_b": inputs_b[sl],
            "mask_a": mask_a[sl],
            "mask_b": mask_b[sl],
            "ident": ident,
            "ident_r": ident,
            "ones": ones,
        })

    res = run_bass_kernel_spmd(nc, in_maps, list(range(N_CORES)))
    a_att = np.concatenate([r["a_att"] for r in res.results], axis=0)
    b_att = np.concatenate([r["b_att"] for r in res.results], axis=0)
    return (a_att, b_att)
